# revision 30
# baseline (speedup 1.0000x reference)
"""Trainium2 Bass kernel for moe_routing (nn_CITADEL_15118284882566).

Math: the reference collapses (qw >= 0; the max rows always contain zeros
from non-matches, so negative branches never survive) to, per pair b:

    out[b] = sum_q qw[b,q] * relu( max_{l,kd} sims[b,q,l] * dw[b,l,kd]
                                   * [d_id[b,l,kd] == q_id[b,q]] )
             + dot(q_cls[b], d_cls[b])

Device strategy (data-parallel over B across 8 cores, 64 pairs/core,
16 groups of 4 pairs; partitions = 4 pairs x 32 queries):

1. DIFF2 = -(d_id - q_id)^2 + 2^-12 * dw via 26-row-stacked f16 matmuls
   (two l-chunks of [128, 5*256] f32 PSUM, double buffered). Ids are split
   into three 5-bit chunks (a,b,c <= 31) so -(d-q)^2 expands into bilinear
   rows whose operands are all fp16-exact integers; PSUM f32 accumulation
   is exact, dw rows accumulate last. Matches give DIFF2 = 2^-12*fp16(dw)
   EXACTLY; non-matches give DIFF2 <= -1 + 2^-12 (strictly negative).
2. ACT relu-copies DIFF2 * 2^20 -> f16 (kd-major d2s layout): matches
   become dw*2^8, non-matches 0. No NaN/inf masking needed anywhere.
3. sims via 4 column-tiled matmuls (contraction over D=128 on partitions);
   qw is pre-folded into qT on the host.
4. kd max-tree (3 DVE ops), prd = sims*dmx, reduce_max over l.
5. Single merged epilogue matmul [e4s|ones].T @ [res|cp] -> tok sums
   (scaled 2^-8) and cls dots in one [5, G+BPC] output, one DMA.

PSUM budget: dfc [128,1280] x2 bufs = 6 banks + sims [128,512] x2 = 8.
"""
import sys

sys.path.insert(0, "/opt/trn_rl_repo")

import numpy as np

B, LQ, LD, KQ, KD, D = 512, 32, 512, 1, 5, 128
NCORES = 8
BPC = B // NCORES          # 64 pairs per core
NB = 4                     # pairs per group
G = BPC // NB              # 16 groups
P = 128
JD = KD * LD               # 2560
LDC = LD // 2              # 256 l per l-chunk
JC = KD * LDC              # 1280 cols per chunk
KSTACK = 26
EPS = 2.0 ** -12
SCALE = 2.0 ** 20

_CACHED = {}

# engine-assignment tuning
# NOTE: Pool (GPSIMD) TENSOR_TENSOR fails the V3 ISA engine check in this
# toolchain — GPSIMD can only do DMA triggering / tensor_copy. Concurrent
# PE row-group matmuls writing the same PSUM bank hang the device.
DEFAULT_OPTS = dict(
    dtq_bufs=3,
    rhx_bufs=4,
    d2s_bufs=2,
    dtq_eng="sync",   # dTq DMA engine: sync | scalar | gpsimd
    rhx_eng="gpsimd", # rhx DMA engine
    rowpack=True,     # 2-way PE row-group packing: row-block = l-chunk, so
                      # concurrent streams write disjoint PSUM tiles/banks
)


def _build_module(**kw):
    opts = dict(DEFAULT_OPTS)
    opts.update(kw)
    import concourse.bacc as bacc
    import concourse.mybir as mybir
    from concourse import tile

    f16 = mybir.dt.float16
    f32 = mybir.dt.float32
    Alu = mybir.AluOpType
    Act = mybir.ActivationFunctionType

    nc = bacc.Bacc("TRN2", target_bir_lowering=False, debug=False)

    def eng(name):
        return {"sync": nc.sync, "scalar": nc.scalar, "gpsimd": nc.gpsimd}[name]

    # fused inputs
    dtq_d = nc.dram_tensor("dtq", [G, D, NB * LD + NB * LQ], f16, kind="ExternalInput")
    rhx_d = nc.dram_tensor("rhx", [G, 2, KSTACK, 128 + JC], f16, kind="ExternalInput")
    epi_d = nc.dram_tensor("epi", [P, 2 * BPC + NB + 1], f32, kind="ExternalInput")

    out_d = nc.dram_tensor("out", [NB + 1, G + BPC], f32, kind="ExternalOutput")

    with tile.TileContext(nc) as tc:
        with (
            tc.tile_pool(name="sb_dtq", bufs=opts["dtq_bufs"]) as sb_dtq,
            tc.tile_pool(name="sb_rhx", bufs=opts["rhx_bufs"]) as sb_rhx,
            tc.tile_pool(name="sb_big", bufs=opts["d2s_bufs"]) as sb_big,
            tc.tile_pool(name="sb_wk", bufs=2) as sb_wk,
            tc.tile_pool(name="sb_res", bufs=1) as sb_res,
            tc.tile_pool(name="ps_diff", bufs=2, space="PSUM") as ps_diff,
            tc.tile_pool(name="ps_s", bufs=2, space="PSUM") as ps_s,
        ):
            # rescp: cols [0, G) = res (per-group relu'd maxes; qw is folded
            # into sims), cols [G, G+BPC) = cp (cls elementwise prods)
            rescp = sb_res.tile([P, G + BPC], f32)
            mxall = sb_res.tile([P, G], f32)
            epi_t = sb_res.tile([P, 2 * BPC + NB + 1], f32)
            nc.sync.dma_start(epi_t[:], epi_d[:])
            # cls products, ready as soon as epi lands
            nc.vector.tensor_tensor(rescp[:, G:], epi_t[:, 0:BPC],
                                    epi_t[:, BPC:2 * BPC], Alu.mult)

            for g in range(G):
                dtq_t = sb_dtq.tile([D, NB * LD + NB * LQ], f16, name="dtq_t")
                rhx_t = sb_rhx.tile([58, 128 + JC], f16, name="rhx_t")
                eng(opts["rhx_eng"]).dma_start(rhx_t[0:KSTACK, :], rhx_d[g, 0])
                eng(opts["rhx_eng"]).dma_start(rhx_t[32:32 + KSTACK, :],
                                               rhx_d[g, 1])
                eng(opts["dtq_eng"]).dma_start(dtq_t[:], dtq_d[g, :, :])

                # d2s column layout is kd-major over full l: col = k*LD + l
                # (l = lc*LDC + j), so every tree slice is 2D-contiguous.
                d2s = sb_big.tile([P, JD], f16, name="d2s")
                d2s_k = d2s.rearrange("p (k j) -> p k j", k=KD)
                # row-block = l-chunk: the two dfc pool slots are disjoint
                # PSUM bank sets, so the two PE row-group streams never
                # write the same bank (same-bank interleave hangs the HW).
                dfc0 = ps_diff.tile([P, JC], f32, name="dfc")
                dfc1 = ps_diff.tile([P, JC], f32, name="dfc")
                for k in range(KD):
                    for lc, dfc in ((0, dfc0), (1, dfc1)):
                        base = 32 * lc if opts["rowpack"] else 0
                        nc.tensor.matmul(
                            dfc[:, k * LDC:(k + 1) * LDC],
                            rhx_t[base:base + KSTACK, 0:P],
                            rhx_t[base:base + KSTACK,
                                  128 + k * LDC: 128 + (k + 1) * LDC],
                            start=True, stop=True,
                        )
                for lc, dfc in ((0, dfc0), (1, dfc1)):
                    nc.scalar.activation(
                        d2s_k[:, :, lc * LDC:(lc + 1) * LDC],
                        dfc.rearrange("p (k j) -> p k j", k=KD),
                        Act.Relu, bias=0.0, scale=SCALE)

                s_ps = ps_s.tile([P, LD], f32, name="s_ps", tag="spool")
                for b in range(NB):
                    nc.tensor.matmul(
                        s_ps[b * LQ:(b + 1) * LQ, :],
                        dtq_t[:, NB * LD + b * LQ: NB * LD + (b + 1) * LQ],
                        dtq_t[:, b * LD:(b + 1) * LD],
                        start=True, stop=True,
                        tile_position=(0, b * LQ),
                    )

                # kd max-tree: 3 ops, all 2D-contiguous
                tA = sb_wk.tile([P, 2 * LD], f16, name="tA")
                nc.vector.tensor_tensor(tA[:], d2s[:, 0:2 * LD],
                                        d2s[:, 2 * LD:4 * LD], Alu.max)
                dmx = sb_wk.tile([P, LD], f16, name="dmx")
                nc.vector.tensor_tensor(dmx[:], tA[:, 0:LD], tA[:, LD:2 * LD],
                                        Alu.max)
                nc.vector.tensor_tensor(dmx[:], dmx[:], d2s[:, 4 * LD:5 * LD],
                                        Alu.max)

                prd = sb_wk.tile([P, LD], f16, name="prd")
                nc.vector.tensor_tensor(prd[:], s_ps[:], dmx[:], Alu.mult)
                nc.vector.reduce_max(mxall[:, g:g + 1], prd[:],
                                     axis=mybir.AxisListType.X)

            # res = max(mxall, 0), batched (carries the 2^8 factor; the
            # epilogue one-hot matmul undoes it with 2^-8)
            nc.vector.tensor_scalar(rescp[:, 0:G], mxall[:], 0.0, None, Alu.max)

            # ---- merged epilogue: [e4s|ones].T @ [res|cp] ----
            out_ps = ps_s.tile([NB + 1, G + BPC], f32, name="out_ps", tag="spool")
            nc.tensor.matmul(out_ps[:], epi_t[:, 2 * BPC:2 * BPC + NB + 1],
                             rescp[:], start=True, stop=True)
            out_sb = sb_res.tile([NB + 1, G + BPC], f32)
            nc.vector.tensor_copy(out_sb[:], out_ps[:])
            nc.sync.dma_start(out_d[:], out_sb[:])

    nc.compile()
    return nc


def _prep_core_inputs(c, q_repr, q_w, q_ids, q_cls, d_repr, d_w, d_ids, d_cls):
    """Pure layout/packing for one core's 64 pairs."""
    s = slice(c * BPC, (c + 1) * BPC)
    qr = q_repr[s]          # [64, 32, 128] f32
    qw = q_w[s, :, 0]       # [64, 32]
    qi = q_ids[s, :, 0]     # [64, 32] int64
    qc = q_cls[s]           # [64, 128]
    dr = d_repr[s]          # [64, 512, 128]
    dw = d_w[s]             # [64, 512, 5]
    di = d_ids[s]           # [64, 512, 5]
    dc = d_cls[s]           # [64, 128]

    # dtq: [G, D, NB*LD + NB*LQ]: dT cols then qTx cols (qw folded into qT)
    dtq = np.empty((G, D, NB * LD + NB * LQ), np.float16)
    dtq[:, :, :NB * LD] = (
        dr.reshape(G, NB, LD, D).transpose(0, 3, 1, 2).reshape(G, D, NB * LD)
    ).astype(np.float16)
    qrw = qr * qw[:, :, None]   # fold qw into the query reprs
    dtq[:, :, NB * LD:] = (
        qrw.reshape(G, NB, LQ, D).transpose(0, 3, 1, 2).reshape(G, D, NB * LQ)
    ).astype(np.float16)

    # 5-bit id chunks (ids < 2^15)
    qa = (qi >> 10).astype(np.float32)
    qb = ((qi >> 5) & 31).astype(np.float32)
    qcq = (qi & 31).astype(np.float32)
    da = (di >> 10).astype(np.float32)
    db = ((di >> 5) & 31).astype(np.float32)
    dcc = (di & 31).astype(np.float32)
    dw16 = dw.astype(np.float16).astype(np.float32)

    E = np.zeros((NB, P), np.float32)
    for b in range(NB):
        E[b, b * LQ:(b + 1) * LQ] = 1.0

    def dcols(x):
        # [G*NB, LD, KD] batch values -> [G, NB, JD] in l-chunk-major
        # column order: j = lc*(KD*LDC) + kd*LDC + (l - lc*LDC)
        return (x.reshape(G, NB, 2, LDC, KD)
                 .transpose(0, 1, 2, 4, 3).reshape(G, NB, JD))

    # full rhs [G, 26, JD] (l-chunk-major cols) + lhsT [G, 26, 128]
    rhs = np.zeros((G, KSTACK, JD), np.float32)
    rhs[:, 0:4] = dcols(da * da + db * db)
    rhs[:, 4:8] = dcols(dcc * dcc)
    rhs[:, 8:12] = dcols(da)
    rhs[:, 12:16] = dcols(db)
    rhs[:, 16:20] = dcols(dcc)
    rhs[:, 20] = 1.0
    rhs[:, 21] = 1.0
    rhs[:, 22:26] = dcols(dw16)

    qar = qa.reshape(G, P)
    qbr = qb.reshape(G, P)
    qcr = qcq.reshape(G, P)
    lhsT = np.zeros((G, KSTACK, P), np.float32)
    lhsT[:, 0:4] = -E
    lhsT[:, 4:8] = -E
    lhsT[:, 8:12] = (2.0 * qar)[:, None, :] * E
    lhsT[:, 12:16] = (2.0 * qbr)[:, None, :] * E
    lhsT[:, 16:20] = (2.0 * qcr)[:, None, :] * E
    lhsT[:, 20] = -(qar * qar + qbr * qbr)
    lhsT[:, 21] = -(qcr * qcr)
    lhsT[:, 22:26] = EPS * E

    # rhx: [G, 2, 26, 128 + JC]: row-block r = l-chunk r's rhs + lhsT copy
    rhx = np.empty((G, 2, KSTACK, 128 + JC), np.float32)
    for r in range(2):
        rhx[:, r, :, 0:P] = lhsT
        rhx[:, r, :, 128:] = rhs[:, :, r * JC:(r + 1) * JC]

    epi = np.zeros((P, 2 * BPC + NB + 1), np.float32)
    epi[:, 0:BPC] = qc.T
    epi[:, BPC:2 * BPC] = dc.T
    for b in range(NB):
        epi[b * LQ:(b + 1) * LQ, 2 * BPC + b] = 2.0 ** -8
    epi[:, 2 * BPC + NB] = 1.0

    return {
        "dtq": dtq,
        "rhx": rhx.astype(np.float16),
        "epi": epi,
    }


def kernel(q_expert_repr, q_expert_weights, q_expert_ids, q_cls_repr,
           d_expert_repr, d_expert_weights, d_expert_ids, d_cls_repr):
    from concourse.bass_utils import run_bass_kernel_spmd

    q_repr = np.asarray(q_expert_repr, np.float32)
    q_w = np.asarray(q_expert_weights, np.float32)
    q_ids = np.asarray(q_expert_ids, np.int64)
    q_cls = np.asarray(q_cls_repr, np.float32)
    d_repr = np.asarray(d_expert_repr, np.float32)
    d_w = np.asarray(d_expert_weights, np.float32)
    d_ids = np.asarray(d_expert_ids, np.int64)
    d_cls = np.asarray(d_cls_repr, np.float32)

    if "nc" not in _CACHED:
        _CACHED["nc"] = _build_module()
    nc = _CACHED["nc"]

    in_maps = [
        _prep_core_inputs(c, q_repr, q_w, q_ids, q_cls, d_repr, d_w, d_ids, d_cls)
        for c in range(NCORES)
    ]
    rr = run_bass_kernel_spmd(nc, in_maps, core_ids=list(range(NCORES)))

    out = np.zeros((B,), np.float32)
    for c in range(NCORES):
        o = rr.results[c]["out"]            # [NB+1, G+BPC]
        tok = o[0:NB, 0:G]                  # [NB, G]
        cls = o[NB, G:]                     # [BPC]
        out[c * BPC:(c + 1) * BPC] = tok.T.reshape(-1) + cls
    return out


if __name__ == "__main__":
    rng = np.random.default_rng(0)
    ins = {
        "q_expert_repr": rng.standard_normal((B, LQ, D)).astype(np.float32),
        "q_expert_weights": rng.random((B, LQ, KQ)).astype(np.float32),
        "q_expert_ids": rng.integers(0, 30522, (B, LQ, KQ)).astype(np.int64),
        "q_cls_repr": rng.standard_normal((B, D)).astype(np.float32),
        "d_expert_repr": rng.standard_normal((B, LD, D)).astype(np.float32),
        "d_expert_weights": rng.random((B, LD, KD)).astype(np.float32),
        "d_expert_ids": rng.integers(0, 30522, (B, LD, KD)).astype(np.int64),
        "d_cls_repr": rng.standard_normal((B, D)).astype(np.float32),
    }
    out = kernel(**ins)
    print("kernel out[:8]:", out[:8])
